# revision 26
# baseline (speedup 1.0000x reference)
"""2D DWT (db2, FFT-equivalent circular conv) as TensorE matmuls on 8 trn2 cores.

Math: for each (b,c) slice X (128x128), with F[k,j] = w[t] at k=(2j+2-t) mod 128
(the circular 4-tap filter + stride-2 decimation as a 128x64 matrix):
    LL = Fl^T X Fl,  LH = Fh^T X Fl,  HL = Fl^T X Fh,  HH = Fh^T X Fh.
With W2 = [Fl | Fh] (128x128):
    stage 1:  out1 = X^T @ W2 = [B_lT | B_hT]           (w on partitions)
    stage 2:  out2 = W2^T @ out1 = [[LL^T, LH^T], [HL^T, HH^T]]
out2 has partitions = j (W-direction output), free = i (H-direction output);
the final transpose of each 64x64 quadrant happens on the host at gather time.

Everything runs in bf16 (inputs, weights, intermediates, outputs) with fp32
PSUM accumulation: the 4-tap filters give short dot products, so quantization
error stays ~6e-3 relative -- well inside the 2e-2 gate -- while halving both
HBM traffic and TensorE column count vs an fp16 hi/lo split.

Sharding: 768 (b,c) slices split contiguously, 96 per core; pure data parallel.
Per-core input shards are transposed on the host to (h, s, w) so every DMA
reads multi-KB contiguous runs per partition. Input DMAs ride the sync ring,
output DMAs the scalar ring, so neither blocks the other at the sequencer.
"""

import numpy as np

_NCORES = 8
_S = 96          # slices per core
_G = 8           # slices per chunk (uniform)
_N = 128

_compiled = None


def _build_w2(w_l: np.ndarray, w_h: np.ndarray) -> np.ndarray:
    W2 = np.zeros((_N, _N), dtype=np.float32)
    for col, w in ((0, w_l), (64, w_h)):
        w = np.asarray(w, dtype=np.float32).reshape(-1)
        L = w.shape[0]
        for j in range(_N // 2):
            for t in range(L):
                W2[(2 * j + L // 2 - t) % _N, col + j] += w[t]
    return W2


def _build_nc():
    import concourse.bacc as bacc
    import concourse.tile as tile
    import concourse.mybir as mybir

    f32 = mybir.dt.float32
    bf16 = mybir.dt.bfloat16
    nc = bacc.Bacc("TRN2", target_bir_lowering=False, debug=False)

    xin = nc.dram_tensor("xin", [_N, _S, _N], bf16, kind="ExternalInput")  # (h, s, w)
    w2_d = nc.dram_tensor("w2", [_N, _N], bf16, kind="ExternalInput")
    out_t = nc.dram_tensor("out_t", [_N, _S, _N], bf16, kind="ExternalOutput")

    # uniform 8-slice chunks: chunk == PSUM group == output store, so the
    # whole kernel is one homogeneous software pipeline with no boundaries
    chunks = [_G] * (_S // _G)
    assert sum(chunks) == _S

    # balance PSUM->SBUF cast copies across ACT (1.2 GHz) and DVE (0.96 GHz)
    # by projected busy time (ns per copy incl. per-op ramp overhead);
    # ACT starts charged for its one-time activation-table load
    est = {"act": 1300.0, "dve": 0.0}

    def pick_engine(cols):
        t_act = cols / 1.2 + 240
        t_dve = cols / 0.96 + 295
        e = "act" if est["act"] + t_act <= est["dve"] + t_dve else "dve"
        est[e] += t_act if e == "act" else t_dve
        return e

    starts = [sum(chunks[:i]) for i in range(len(chunks))]
    LOOKAHEAD = 4

    with tile.TileContext(nc) as tc:
        with (
            tc.tile_pool(name="singles", bufs=1) as singles,
            tc.tile_pool(name="xin", bufs=4) as xinp,
            tc.tile_pool(name="mid", bufs=3) as mid,
            tc.tile_pool(name="out", bufs=3) as outp,
            tc.tile_pool(name="ps1", bufs=2, space="PSUM") as ps1p,
            tc.tile_pool(name="ps2", bufs=2, space="PSUM") as ps2p,
        ):
            w2_sb = singles.tile([_N, _N], bf16)
            # tiny (32 KB) weight transfer rides the ACT HWDGE ring so the
            # sync ring leads with input chunks
            nc.scalar.dma_start(out=w2_sb[:], in_=w2_d[:])

            # PE pre-warm: the HAM clock gate holds the PE at 1.2 GHz until
            # it has been busy ~3.4us. The PE is idle during the DMA ramp
            # anyway, so a throwaway matmul stream against a zeroed tile
            # brings it to 2.4 GHz by the time the first real chunk lands.
            warm_sb = singles.tile([_N, 256], bf16)
            nc.gpsimd.memset(warm_sb[:], 0)
            # the warm tile doubles as chunk-0's first stage-1 PSUM tile so
            # the pool high-water stays within the 8 PSUM banks
            warm_ps = ps1p.tile([_N, 1024], f32, name="ps1")
            for _ in range(14):
                nc.tensor.matmul(
                    warm_ps[:, :256],
                    lhsT=warm_sb[:, :_N],
                    rhs=warm_sb[:],
                    start=True,
                    stop=True,
                )

            x_tiles = {}

            def emit_in(k):
                G, cc = chunks[k], starts[k]
                t = xinp.tile([_N, _G * _N], bf16, tag="x")
                nc.sync.dma_start(
                    out=t[:, : G * _N].rearrange("p (s w) -> p s w", s=G),
                    in_=xin[:, cc : cc + G, :],
                )
                x_tiles[k] = t

            for k in range(min(LOOKAHEAD, len(chunks))):
                emit_in(k)

            for ci, G in enumerate(chunks):
                c0 = starts[ci]
                x_sb = x_tiles.pop(ci)
                y_sb = mid.tile([_N, _G * _N], bf16, tag="mid")
                for q in range((G + 7) // 8):
                    # 8 slices' stage-1 results fill a 2-bank PSUM tile;
                    # one wide cast copy amortizes the engine ramp latency
                    kn = min(8, G - q * 8)
                    if ci == 0 and q == 0:
                        ps1 = warm_ps
                    else:
                        ps1 = ps1p.tile([_N, 1024], f32)
                    for k in range(kn):
                        s = q * 8 + k
                        nc.tensor.matmul(
                            ps1[:, k * _N : (k + 1) * _N],
                            lhsT=x_sb[:, s * _N : (s + 1) * _N],
                            rhs=w2_sb[:],
                            start=True,
                            stop=True,
                        )
                    dst = y_sb[:, q * 1024 : q * 1024 + kn * _N]
                    if pick_engine(kn * _N) == "act":
                        nc.scalar.copy(out=dst, in_=ps1[:, : kn * _N])
                    else:
                        nc.vector.tensor_copy(dst, ps1[:, : kn * _N])

                if ci + LOOKAHEAD < len(chunks):
                    emit_in(ci + LOOKAHEAD)

                # stage 2 + output: one 8-slice group at a time, each group's
                # 256 KB store issued as soon as its cast lands, so the
                # output stream tracks the casts instead of whole chunks
                out2_sb = outp.tile([_N, _G * _N], bf16, tag="out")
                for g in range((G * _N + 1023) // 1024):
                    g0 = g * 1024
                    gw = min(1024, G * _N - g0)
                    gn = gw // _N
                    ps2 = ps2p.tile([_N, 1024], f32)
                    for h in range((gw + 511) // 512):
                        hw = min(512, gw - h * 512)
                        nc.tensor.matmul(
                            ps2[:, h * 512 : h * 512 + hw],
                            lhsT=w2_sb[:],
                            rhs=y_sb[:, g0 + h * 512 : g0 + h * 512 + hw],
                            start=True,
                            stop=True,
                        )
                    dst = out2_sb[:, g0 : g0 + gw]
                    if pick_engine(gw) == "act":
                        nc.scalar.copy(out=dst, in_=ps2[:, :gw])
                    else:
                        nc.vector.tensor_copy(dst, ps2[:, :gw])
                    nc.sync.dma_start(
                        out=out_t[:, c0 + g * 8 : c0 + g * 8 + gn, :],
                        in_=out2_sb[:, g0 : g0 + gw].rearrange(
                            "p (s f) -> p s f", s=gn
                        ),
                    )
    nc.finalize()
    return nc


def _get_compiled():
    global _compiled
    if _compiled is None:
        _compiled = _build_nc()
    return _compiled


def run_on_hw(x: np.ndarray, w_l: np.ndarray, w_h: np.ndarray, trace: bool = False):
    """Returns ((LL, LH, HL, HH), exec_time_ns or None)."""
    import ml_dtypes
    from concourse.bass_utils import run_bass_kernel_spmd

    bf16 = ml_dtypes.bfloat16
    x = np.asarray(x, dtype=np.float32)
    W2 = _build_w2(np.asarray(w_l), np.asarray(w_h)).astype(bf16)

    xf = x.reshape(-1, _N, _N)  # (768, 128, 128)
    nc = _get_compiled()
    in_maps = []
    for i in range(_NCORES):
        shard = xf[i * _S : (i + 1) * _S].transpose(1, 0, 2).astype(bf16)
        in_maps.append({"xin": np.ascontiguousarray(shard), "w2": W2})
    res = run_bass_kernel_spmd(nc, in_maps, list(range(_NCORES)), trace=trace)

    quads = [[], [], [], []]  # LL, LH, HL, HH per-core chunks, each (S, 64, 64)
    for i in range(_NCORES):
        ot = res.results[i]["out_t"]  # (128, 96, 128) = [j(+64*qr), s, i(+64*qc)]
        quads[0].append(np.transpose(ot[0:64, :, 0:64], (1, 2, 0)))
        quads[1].append(np.transpose(ot[0:64, :, 64:128], (1, 2, 0)))
        quads[2].append(np.transpose(ot[64:128, :, 0:64], (1, 2, 0)))
        quads[3].append(np.transpose(ot[64:128, :, 64:128], (1, 2, 0)))

    B, C, H, W = x.shape
    out = tuple(
        np.ascontiguousarray(np.concatenate(q, axis=0))
        .reshape(B, C, H // 2, W // 2)
        .astype(np.float32)
        for q in quads
    )
    return out, res.exec_time_ns


def kernel(x: np.ndarray, w_l: np.ndarray, w_h: np.ndarray):
    out, _ = run_on_hw(x, w_l, w_h, trace=False)
    return out


# revision 31
# speedup vs baseline: 1.0923x; 1.0923x over previous
"""2D DWT (db2, FFT-equivalent circular conv) as TensorE matmuls on 8 trn2 cores.

Math: for each (b,c) slice X (128x128), with F[k,j] = w[t] at k=(2j+2-t) mod 128
(the circular 4-tap filter + stride-2 decimation as a 128x64 matrix):
    LL = Fl^T X Fl,  LH = Fh^T X Fl,  HL = Fl^T X Fh,  HH = Fh^T X Fh.
With W2 = [Fl | Fh] (128x128):
    stage 1:  out1 = X^T @ W2 = [B_lT | B_hT]           (w on partitions)
    stage 2:  out2 = W2^T @ out1 = [[LL^T, LH^T], [HL^T, HH^T]]
out2 has partitions = j (W-direction output), free = i (H-direction output);
the final transpose of each 64x64 quadrant happens on the host at gather time.

Everything runs in bf16 (inputs, weights, intermediates, outputs) with fp32
PSUM accumulation: the 4-tap filters give short dot products, so quantization
error stays ~6e-3 relative -- well inside the 2e-2 gate -- while halving both
HBM traffic and TensorE column count vs an fp16 hi/lo split.

Sharding: 768 (b,c) slices split contiguously, 96 per core; pure data parallel.
Per-core input shards are transposed on the host to (h, s, w) so every DMA
reads multi-KB contiguous runs per partition. Input DMAs ride the sync ring,
output DMAs the scalar ring, so neither blocks the other at the sequencer.
"""

import numpy as np

_NCORES = 8
_S = 96          # slices per core
_G = 24          # max slices per chunk
_N = 128

_compiled = None


def _build_w2(w_l: np.ndarray, w_h: np.ndarray) -> np.ndarray:
    W2 = np.zeros((_N, _N), dtype=np.float32)
    for col, w in ((0, w_l), (64, w_h)):
        w = np.asarray(w, dtype=np.float32).reshape(-1)
        L = w.shape[0]
        for j in range(_N // 2):
            for t in range(L):
                W2[(2 * j + L // 2 - t) % _N, col + j] += w[t]
    return W2


def _build_nc():
    import concourse.bacc as bacc
    import concourse.tile as tile
    import concourse.mybir as mybir

    f32 = mybir.dt.float32
    bf16 = mybir.dt.bfloat16
    nc = bacc.Bacc("TRN2", target_bir_lowering=False, debug=False)

    xin = nc.dram_tensor("xin", [_N, _S, _N], bf16, kind="ExternalInput")  # (h, s, w)
    w2_d = nc.dram_tensor("w2", [_N, _N], bf16, kind="ExternalInput")
    out_t = nc.dram_tensor("out_t", [_N, _S, _N], bf16, kind="ExternalOutput")

    # graduated chunks: small at start (compute starts early) and end (short
    # serial tail); sizes that keep stage-1 PSUM groups (<=8 slices) efficient
    chunks = [8, 16, 24, 24, 16, 8]
    assert sum(chunks) == _S

    # balance PSUM->SBUF cast copies across ACT (1.2 GHz) and DVE (0.96 GHz)
    # by projected busy time (ns per copy incl. per-op ramp overhead);
    # ACT starts charged for its one-time activation-table load
    est = {"act": 1300.0, "dve": 0.0}

    def pick_engine(cols):
        t_act = cols / 1.2 + 240
        t_dve = cols / 0.96 + 295
        e = "act" if est["act"] + t_act <= est["dve"] + t_dve else "dve"
        est[e] += t_act if e == "act" else t_dve
        return e

    starts = [sum(chunks[:i]) for i in range(len(chunks))]
    LOOKAHEAD = 3

    with tile.TileContext(nc) as tc:
        with (
            tc.tile_pool(name="singles", bufs=1) as singles,
            tc.tile_pool(name="xin", bufs=3) as xinp,
            tc.tile_pool(name="mid", bufs=3) as mid,
            tc.tile_pool(name="out", bufs=3) as outp,
            tc.tile_pool(name="ps1", bufs=2, space="PSUM") as ps1p,
            tc.tile_pool(name="ps2", bufs=2, space="PSUM") as ps2p,
        ):
            w2_sb = singles.tile([_N, _N], bf16)
            # tiny (32 KB) weight transfer rides the ACT HWDGE ring so the
            # sync ring leads with input chunks
            nc.scalar.dma_start(out=w2_sb[:], in_=w2_d[:])

            # PE pre-warm: the HAM clock gate holds the PE at 1.2 GHz until
            # it has been busy ~3.4us. The PE is idle during the DMA ramp
            # anyway, so a throwaway matmul stream against a zeroed tile
            # brings it to 2.4 GHz by the time the first real chunk lands.
            warm_sb = singles.tile([_N, 256], bf16)
            nc.gpsimd.memset(warm_sb[:], 0)
            # the warm tile doubles as chunk-0's first stage-1 PSUM tile so
            # the pool high-water stays within the 8 PSUM banks
            warm_ps = ps1p.tile([_N, 1024], f32, name="ps1")
            for _ in range(14):
                nc.tensor.matmul(
                    warm_ps[:, :256],
                    lhsT=warm_sb[:, :_N],
                    rhs=warm_sb[:],
                    start=True,
                    stop=True,
                )

            x_tiles = {}

            def emit_in(k):
                G, cc = chunks[k], starts[k]
                t = xinp.tile([_N, _G * _N], bf16, tag="x")
                nc.sync.dma_start(
                    out=t[:, : G * _N].rearrange("p (s w) -> p s w", s=G),
                    in_=xin[:, cc : cc + G, :],
                )
                x_tiles[k] = t

            for k in range(min(LOOKAHEAD, len(chunks))):
                emit_in(k)

            for ci, G in enumerate(chunks):
                c0 = starts[ci]
                x_sb = x_tiles.pop(ci)
                y_sb = mid.tile([_N, _G * _N], bf16, tag="mid")
                for q in range((G + 7) // 8):
                    # 8 slices' stage-1 results fill a 2-bank PSUM tile;
                    # one wide cast copy amortizes the engine ramp latency
                    kn = min(8, G - q * 8)
                    if ci == 0 and q == 0:
                        ps1 = warm_ps
                    else:
                        ps1 = ps1p.tile([_N, 1024], f32)
                    for k in range(kn):
                        s = q * 8 + k
                        nc.tensor.matmul(
                            ps1[:, k * _N : (k + 1) * _N],
                            lhsT=x_sb[:, s * _N : (s + 1) * _N],
                            rhs=w2_sb[:],
                            start=True,
                            stop=True,
                        )
                    dst = y_sb[:, q * 1024 : q * 1024 + kn * _N]
                    if pick_engine(kn * _N) == "act":
                        nc.scalar.copy(out=dst, in_=ps1[:, : kn * _N])
                    else:
                        nc.vector.tensor_copy(dst, ps1[:, : kn * _N])

                if ci + LOOKAHEAD < len(chunks):
                    emit_in(ci + LOOKAHEAD)

                out2_sb = outp.tile([_N, _G * _N], bf16, tag="out")
                for g in range((G * _N + 1023) // 1024):
                    g0 = g * 1024
                    gw = min(1024, G * _N - g0)
                    ps2 = ps2p.tile([_N, 1024], f32)
                    for h in range((gw + 511) // 512):
                        hw = min(512, gw - h * 512)
                        nc.tensor.matmul(
                            ps2[:, h * 512 : h * 512 + hw],
                            lhsT=w2_sb[:],
                            rhs=y_sb[:, g0 + h * 512 : g0 + h * 512 + hw],
                            start=True,
                            stop=True,
                        )
                    dst = out2_sb[:, g0 : g0 + gw]
                    if pick_engine(gw) == "act":
                        nc.scalar.copy(out=dst, in_=ps2[:, :gw])
                    else:
                        nc.vector.tensor_copy(dst, ps2[:, :gw])

                nc.sync.dma_start(
                    out=out_t[:, c0 : c0 + G, :],
                    in_=out2_sb[:, : G * _N].rearrange("p (s f) -> p s f", s=G),
                )
    nc.finalize()
    return nc


def _get_compiled():
    global _compiled
    if _compiled is None:
        _compiled = _build_nc()
    return _compiled


def run_on_hw(x: np.ndarray, w_l: np.ndarray, w_h: np.ndarray, trace: bool = False):
    """Returns ((LL, LH, HL, HH), exec_time_ns or None)."""
    import ml_dtypes
    from concourse.bass_utils import run_bass_kernel_spmd

    bf16 = ml_dtypes.bfloat16
    x = np.asarray(x, dtype=np.float32)
    W2 = _build_w2(np.asarray(w_l), np.asarray(w_h)).astype(bf16)

    xf = x.reshape(-1, _N, _N)  # (768, 128, 128)
    nc = _get_compiled()
    in_maps = []
    for i in range(_NCORES):
        shard = xf[i * _S : (i + 1) * _S].transpose(1, 0, 2).astype(bf16)
        in_maps.append({"xin": np.ascontiguousarray(shard), "w2": W2})
    res = run_bass_kernel_spmd(nc, in_maps, list(range(_NCORES)), trace=trace)

    quads = [[], [], [], []]  # LL, LH, HL, HH per-core chunks, each (S, 64, 64)
    for i in range(_NCORES):
        ot = res.results[i]["out_t"]  # (128, 96, 128) = [j(+64*qr), s, i(+64*qc)]
        quads[0].append(np.transpose(ot[0:64, :, 0:64], (1, 2, 0)))
        quads[1].append(np.transpose(ot[0:64, :, 64:128], (1, 2, 0)))
        quads[2].append(np.transpose(ot[64:128, :, 0:64], (1, 2, 0)))
        quads[3].append(np.transpose(ot[64:128, :, 64:128], (1, 2, 0)))

    B, C, H, W = x.shape
    out = tuple(
        np.ascontiguousarray(np.concatenate(q, axis=0))
        .reshape(B, C, H // 2, W // 2)
        .astype(np.float32)
        for q in quads
    )
    return out, res.exec_time_ns


def kernel(x: np.ndarray, w_l: np.ndarray, w_h: np.ndarray):
    out, _ = run_on_hw(x, w_l, w_h, trace=False)
    return out


# revision 36
# speedup vs baseline: 1.0978x; 1.0050x over previous
"""2D DWT (db2, FFT-equivalent circular conv) as TensorE matmuls on 8 trn2 cores.

Math: for each (b,c) slice X (128x128), with F[k,j] = w[t] at k=(2j+2-t) mod 128
(the circular 4-tap filter + stride-2 decimation as a 128x64 matrix):
    LL = Fl^T X Fl,  LH = Fh^T X Fl,  HL = Fl^T X Fh,  HH = Fh^T X Fh.
With W2 = [Fl | Fh] (128x128):
    stage 1:  out1 = X^T @ W2 = [B_lT | B_hT]           (w on partitions)
    stage 2:  out2 = W2^T @ out1 = [[LL^T, LH^T], [HL^T, HH^T]]
out2 has partitions = j (W-direction output), free = i (H-direction output);
the final transpose of each 64x64 quadrant happens on the host at gather time.

Everything runs in bf16 (inputs, weights, intermediates, outputs) with fp32
PSUM accumulation: the 4-tap filters give short dot products, so quantization
error stays ~6e-3 relative -- well inside the 2e-2 gate -- while halving both
HBM traffic and TensorE column count vs an fp16 hi/lo split.

Sharding: 768 (b,c) slices split contiguously, 96 per core; pure data parallel.
Per-core input shards are transposed on the host to (h, s, w) so every DMA
reads multi-KB contiguous runs per partition. Input DMAs ride the sync ring,
output DMAs the scalar ring, so neither blocks the other at the sequencer.
"""

import numpy as np

_NCORES = 8
_S = 96          # slices per core
_G = 24          # max slices per chunk
_N = 128

_compiled = None


def _build_w2(w_l: np.ndarray, w_h: np.ndarray) -> np.ndarray:
    W2 = np.zeros((_N, _N), dtype=np.float32)
    for col, w in ((0, w_l), (64, w_h)):
        w = np.asarray(w, dtype=np.float32).reshape(-1)
        L = w.shape[0]
        for j in range(_N // 2):
            for t in range(L):
                W2[(2 * j + L // 2 - t) % _N, col + j] += w[t]
    return W2


def _build_nc():
    import concourse.bacc as bacc
    import concourse.tile as tile
    import concourse.mybir as mybir

    f32 = mybir.dt.float32
    bf16 = mybir.dt.bfloat16
    nc = bacc.Bacc("TRN2", target_bir_lowering=False, debug=False)

    xin = nc.dram_tensor("xin", [_N, _S, _N], bf16, kind="ExternalInput")  # (h, s, w)
    w2_d = nc.dram_tensor("w2", [_N, _N], bf16, kind="ExternalInput")
    out_t = nc.dram_tensor("out_t", [_N, _S, _N], bf16, kind="ExternalOutput")

    # graduated chunks: small at start (compute starts early) and end (short
    # serial tail); sizes that keep stage-1 PSUM groups (<=8 slices) efficient
    chunks = [8, 16, 24, 24, 16, 8]
    assert sum(chunks) == _S

    # PSUM->SBUF cast copies alternate anti-phase across ACT and DVE: group
    # i's stage-1 and stage-2 casts land on OPPOSITE engines, and the
    # assignment swaps every group, so while one group's stage-2 matmuls run
    # the other engine is already casting the next group's stage-1 output
    def pick_engine(i, stage):
        return ("act", "dve")[(i + (0 if stage == 1 else 1)) % 2]

    starts = [sum(chunks[:i]) for i in range(len(chunks))]
    LOOKAHEAD = 4

    with tile.TileContext(nc) as tc:
        with (
            tc.tile_pool(name="singles", bufs=1) as singles,
            tc.tile_pool(name="xin", bufs=3) as xinp,
            tc.tile_pool(name="mid", bufs=3) as mid,
            tc.tile_pool(name="out", bufs=3) as outp,
            tc.tile_pool(name="ps1", bufs=2, space="PSUM") as ps1p,
            tc.tile_pool(name="ps2", bufs=2, space="PSUM") as ps2p,
        ):
            w2_sb = singles.tile([_N, _N], bf16)
            # tiny (32 KB) weight transfer rides the ACT HWDGE ring so the
            # sync ring leads with input chunks
            nc.scalar.dma_start(out=w2_sb[:], in_=w2_d[:])

            # PE pre-warm: the HAM clock gate holds the PE at 1.2 GHz until
            # it has been busy ~3.4us. The PE is idle during the DMA ramp
            # anyway, so a throwaway matmul stream against a zeroed tile
            # brings it to 2.4 GHz by the time the first real chunk lands.
            warm_sb = singles.tile([_N, 256], bf16)
            nc.gpsimd.memset(warm_sb[:], 0)
            # the warm tile doubles as chunk-0's first stage-1 PSUM tile so
            # the pool high-water stays within the 8 PSUM banks
            warm_ps = ps1p.tile([_N, 1024], f32, name="ps1")
            for _ in range(14):
                nc.tensor.matmul(
                    warm_ps[:, :256],
                    lhsT=warm_sb[:, :_N],
                    rhs=warm_sb[:],
                    start=True,
                    stop=True,
                )

            x_tiles = {}

            def emit_in(k):
                G, cc = chunks[k], starts[k]
                t = xinp.tile([_N, _G * _N], bf16, tag="x")
                nc.sync.dma_start(
                    out=t[:, : G * _N].rearrange("p (s w) -> p s w", s=G),
                    in_=xin[:, cc : cc + G, :],
                )
                x_tiles[k] = t

            for k in range(min(LOOKAHEAD, len(chunks))):
                emit_in(k)

            gbase = 0
            for ci, G in enumerate(chunks):
                c0 = starts[ci]
                x_sb = x_tiles.pop(ci)
                y_sb = mid.tile([_N, _G * _N], bf16, tag="mid")
                for q in range((G + 7) // 8):
                    # 8 slices' stage-1 results fill a 2-bank PSUM tile;
                    # one wide cast copy amortizes the engine ramp latency
                    kn = min(8, G - q * 8)
                    if ci == 0 and q == 0:
                        ps1 = warm_ps
                    else:
                        ps1 = ps1p.tile([_N, 1024], f32)
                    for k in range(kn):
                        s = q * 8 + k
                        nc.tensor.matmul(
                            ps1[:, k * _N : (k + 1) * _N],
                            lhsT=x_sb[:, s * _N : (s + 1) * _N],
                            rhs=w2_sb[:],
                            start=True,
                            stop=True,
                        )
                    dst = y_sb[:, q * 1024 : q * 1024 + kn * _N]
                    if pick_engine(gbase + q, 1) == "act":
                        nc.scalar.copy(out=dst, in_=ps1[:, : kn * _N])
                    else:
                        nc.vector.tensor_copy(dst, ps1[:, : kn * _N])

                if ci + LOOKAHEAD < len(chunks):
                    emit_in(ci + LOOKAHEAD)

                out2_sb = outp.tile([_N, _G * _N], bf16, tag="out")
                for g in range((G * _N + 1023) // 1024):
                    g0 = g * 1024
                    gw = min(1024, G * _N - g0)
                    ps2 = ps2p.tile([_N, 1024], f32)
                    for h in range((gw + 511) // 512):
                        hw = min(512, gw - h * 512)
                        nc.tensor.matmul(
                            ps2[:, h * 512 : h * 512 + hw],
                            lhsT=w2_sb[:],
                            rhs=y_sb[:, g0 + h * 512 : g0 + h * 512 + hw],
                            start=True,
                            stop=True,
                        )
                    dst = out2_sb[:, g0 : g0 + gw]
                    if pick_engine(gbase + g, 2) == "act":
                        nc.scalar.copy(out=dst, in_=ps2[:, :gw])
                    else:
                        nc.vector.tensor_copy(dst, ps2[:, :gw])

                gbase += (G + 7) // 8
                nc.sync.dma_start(
                    out=out_t[:, c0 : c0 + G, :],
                    in_=out2_sb[:, : G * _N].rearrange("p (s f) -> p s f", s=G),
                )
    nc.finalize()
    return nc


def _get_compiled():
    global _compiled
    if _compiled is None:
        _compiled = _build_nc()
    return _compiled


def run_on_hw(x: np.ndarray, w_l: np.ndarray, w_h: np.ndarray, trace: bool = False):
    """Returns ((LL, LH, HL, HH), exec_time_ns or None)."""
    import ml_dtypes
    from concourse.bass_utils import run_bass_kernel_spmd

    bf16 = ml_dtypes.bfloat16
    x = np.asarray(x, dtype=np.float32)
    W2 = _build_w2(np.asarray(w_l), np.asarray(w_h)).astype(bf16)

    xf = x.reshape(-1, _N, _N)  # (768, 128, 128)
    nc = _get_compiled()
    in_maps = []
    for i in range(_NCORES):
        shard = xf[i * _S : (i + 1) * _S].transpose(1, 0, 2).astype(bf16)
        in_maps.append({"xin": np.ascontiguousarray(shard), "w2": W2})
    res = run_bass_kernel_spmd(nc, in_maps, list(range(_NCORES)), trace=trace)

    quads = [[], [], [], []]  # LL, LH, HL, HH per-core chunks, each (S, 64, 64)
    for i in range(_NCORES):
        ot = res.results[i]["out_t"]  # (128, 96, 128) = [j(+64*qr), s, i(+64*qc)]
        quads[0].append(np.transpose(ot[0:64, :, 0:64], (1, 2, 0)))
        quads[1].append(np.transpose(ot[0:64, :, 64:128], (1, 2, 0)))
        quads[2].append(np.transpose(ot[64:128, :, 0:64], (1, 2, 0)))
        quads[3].append(np.transpose(ot[64:128, :, 64:128], (1, 2, 0)))

    B, C, H, W = x.shape
    out = tuple(
        np.ascontiguousarray(np.concatenate(q, axis=0))
        .reshape(B, C, H // 2, W // 2)
        .astype(np.float32)
        for q in quads
    )
    return out, res.exec_time_ns


def kernel(x: np.ndarray, w_l: np.ndarray, w_h: np.ndarray):
    out, _ = run_on_hw(x, w_l, w_h, trace=False)
    return out


# revision 37
# speedup vs baseline: 1.1231x; 1.0230x over previous
"""2D DWT (db2, FFT-equivalent circular conv) as TensorE matmuls on 8 trn2 cores.

Math: for each (b,c) slice X (128x128), with F[k,j] = w[t] at k=(2j+2-t) mod 128
(the circular 4-tap filter + stride-2 decimation as a 128x64 matrix):
    LL = Fl^T X Fl,  LH = Fh^T X Fl,  HL = Fl^T X Fh,  HH = Fh^T X Fh.
With W2 = [Fl | Fh] (128x128):
    stage 1:  out1 = X^T @ W2 = [B_lT | B_hT]           (w on partitions)
    stage 2:  out2 = W2^T @ out1 = [[LL^T, LH^T], [HL^T, HH^T]]
out2 has partitions = j (W-direction output), free = i (H-direction output);
the final transpose of each 64x64 quadrant happens on the host at gather time.

Everything runs in bf16 (inputs, weights, intermediates, outputs) with fp32
PSUM accumulation: the 4-tap filters give short dot products, so quantization
error stays ~6e-3 relative -- well inside the 2e-2 gate -- while halving both
HBM traffic and TensorE column count vs an fp16 hi/lo split.

Sharding: 768 (b,c) slices split contiguously, 96 per core; pure data parallel.
Per-core input shards are transposed on the host to (h, s, w) so every DMA
reads multi-KB contiguous runs per partition. Input DMAs ride the sync ring,
output DMAs the scalar ring, so neither blocks the other at the sequencer.
"""

import numpy as np

_NCORES = 8
_S = 96          # slices per core
_G = 24          # max slices per chunk
_N = 128

_compiled = None


def _build_w2(w_l: np.ndarray, w_h: np.ndarray) -> np.ndarray:
    W2 = np.zeros((_N, _N), dtype=np.float32)
    for col, w in ((0, w_l), (64, w_h)):
        w = np.asarray(w, dtype=np.float32).reshape(-1)
        L = w.shape[0]
        for j in range(_N // 2):
            for t in range(L):
                W2[(2 * j + L // 2 - t) % _N, col + j] += w[t]
    return W2


def _build_nc():
    import concourse.bacc as bacc
    import concourse.tile as tile
    import concourse.mybir as mybir

    f32 = mybir.dt.float32
    bf16 = mybir.dt.bfloat16
    nc = bacc.Bacc("TRN2", target_bir_lowering=False, debug=False)

    xin = nc.dram_tensor("xin", [_N, _S, _N], bf16, kind="ExternalInput")  # (h, s, w)
    w2_d = nc.dram_tensor("w2", [_N, _N], bf16, kind="ExternalInput")
    out_t = nc.dram_tensor("out_t", [_N, _S, _N], bf16, kind="ExternalOutput")

    # graduated chunks: small at start (compute starts early) and end (short
    # serial tail); sizes that keep stage-1 PSUM groups (<=8 slices) efficient
    chunks = [8, 16, 24, 24, 16, 8]
    assert sum(chunks) == _S

    # PSUM->SBUF cast copies alternate anti-phase across ACT and DVE: group
    # i's stage-1 and stage-2 casts land on OPPOSITE engines, and the
    # assignment swaps every group, so while one group's stage-2 matmuls run
    # the other engine is already casting the next group's stage-1 output
    def pick_engine(i, stage):
        return ("act", "dve")[(i + (0 if stage == 1 else 1)) % 2]

    starts = [sum(chunks[:i]) for i in range(len(chunks))]
    LOOKAHEAD = 4

    with tile.TileContext(nc) as tc:
        with (
            tc.tile_pool(name="singles", bufs=1) as singles,
            tc.tile_pool(name="xin", bufs=3) as xinp,
            tc.tile_pool(name="mid", bufs=3) as mid,
            tc.tile_pool(name="out", bufs=3) as outp,
            tc.tile_pool(name="ps1", bufs=2, space="PSUM") as ps1p,
            tc.tile_pool(name="ps2", bufs=2, space="PSUM") as ps2p,
        ):
            w2_sb = singles.tile([_N, _N], bf16)
            # tiny (32 KB) weight transfer rides the ACT HWDGE ring so the
            # sync ring leads with input chunks
            nc.scalar.dma_start(out=w2_sb[:], in_=w2_d[:])

            # PE pre-warm: the HAM clock gate holds the PE at 1.2 GHz until
            # it has been busy ~3.4us. The PE is idle during the DMA ramp
            # anyway, so a throwaway matmul stream against a zeroed tile
            # brings it to 2.4 GHz by the time the first real chunk lands.
            warm_sb = singles.tile([_N, 256], bf16)
            nc.gpsimd.memset(warm_sb[:], 0)
            # the warm tile doubles as chunk-0's first stage-1 PSUM tile so
            # the pool high-water stays within the 8 PSUM banks
            warm_ps = ps1p.tile([_N, 1024], f32, name="ps1")
            for _ in range(14):
                nc.tensor.matmul(
                    warm_ps[:, :256],
                    lhsT=warm_sb[:, :_N],
                    rhs=warm_sb[:],
                    start=True,
                    stop=True,
                )

            x_tiles = {}

            def emit_in(k):
                G, cc = chunks[k], starts[k]
                t = xinp.tile([_N, _G * _N], bf16, tag="x")
                nc.sync.dma_start(
                    out=t[:, : G * _N].rearrange("p (s w) -> p s w", s=G),
                    in_=xin[:, cc : cc + G, :],
                )
                x_tiles[k] = t

            for k in range(min(LOOKAHEAD, len(chunks))):
                emit_in(k)

            gbase = 0
            for ci, G in enumerate(chunks):
                c0 = starts[ci]
                x_sb = x_tiles.pop(ci)
                y_sb = mid.tile([_N, _G * _N], bf16, tag="mid")
                for q in range((G + 7) // 8):
                    # 8 slices' stage-1 results fill a 2-bank PSUM tile;
                    # one wide cast copy amortizes the engine ramp latency
                    kn = min(8, G - q * 8)
                    if ci == 0 and q == 0:
                        ps1 = warm_ps
                    else:
                        ps1 = ps1p.tile([_N, 1024], f32)
                    for k in range(kn):
                        s = q * 8 + k
                        nc.tensor.matmul(
                            ps1[:, k * _N : (k + 1) * _N],
                            lhsT=x_sb[:, s * _N : (s + 1) * _N],
                            rhs=w2_sb[:],
                            start=True,
                            stop=True,
                        )
                    dst = y_sb[:, q * 1024 : q * 1024 + kn * _N]
                    if pick_engine(gbase + q, 1) == "act":
                        nc.scalar.copy(out=dst, in_=ps1[:, : kn * _N])
                    else:
                        nc.vector.tensor_copy(dst, ps1[:, : kn * _N])

                if ci + LOOKAHEAD < len(chunks):
                    emit_in(ci + LOOKAHEAD)

                out2_sb = outp.tile([_N, _G * _N], bf16, tag="out")
                for g in range((G * _N + 1023) // 1024):
                    g0 = g * 1024
                    gw = min(1024, G * _N - g0)
                    ps2 = ps2p.tile([_N, 1024], f32)
                    for h in range((gw + 511) // 512):
                        hw = min(512, gw - h * 512)
                        nc.tensor.matmul(
                            ps2[:, h * 512 : h * 512 + hw],
                            lhsT=w2_sb[:],
                            rhs=y_sb[:, g0 + h * 512 : g0 + h * 512 + hw],
                            start=True,
                            stop=True,
                        )
                    dst = out2_sb[:, g0 : g0 + gw]
                    if pick_engine(gbase + g, 2) == "act":
                        nc.scalar.copy(out=dst, in_=ps2[:, :gw])
                    else:
                        nc.vector.tensor_copy(dst, ps2[:, :gw])
                    # each 8-slice group's 256 KB store goes out on GpSimd's
                    # SWDGE path the moment its cast lands: the sync ring
                    # carries only inputs (never head-of-line blocked) and
                    # POOL, otherwise idle, absorbs the issue cost
                    gn = gw // _N
                    nc.gpsimd.dma_start(
                        out=out_t[:, c0 + g * 8 : c0 + g * 8 + gn, :],
                        in_=out2_sb[:, g0 : g0 + gw].rearrange(
                            "p (s f) -> p s f", s=gn
                        ),
                    )

                gbase += (G + 7) // 8
    nc.finalize()
    return nc


def _get_compiled():
    global _compiled
    if _compiled is None:
        _compiled = _build_nc()
    return _compiled


def run_on_hw(x: np.ndarray, w_l: np.ndarray, w_h: np.ndarray, trace: bool = False):
    """Returns ((LL, LH, HL, HH), exec_time_ns or None)."""
    import ml_dtypes
    from concourse.bass_utils import run_bass_kernel_spmd

    bf16 = ml_dtypes.bfloat16
    x = np.asarray(x, dtype=np.float32)
    W2 = _build_w2(np.asarray(w_l), np.asarray(w_h)).astype(bf16)

    xf = x.reshape(-1, _N, _N)  # (768, 128, 128)
    nc = _get_compiled()
    in_maps = []
    for i in range(_NCORES):
        shard = xf[i * _S : (i + 1) * _S].transpose(1, 0, 2).astype(bf16)
        in_maps.append({"xin": np.ascontiguousarray(shard), "w2": W2})
    res = run_bass_kernel_spmd(nc, in_maps, list(range(_NCORES)), trace=trace)

    quads = [[], [], [], []]  # LL, LH, HL, HH per-core chunks, each (S, 64, 64)
    for i in range(_NCORES):
        ot = res.results[i]["out_t"]  # (128, 96, 128) = [j(+64*qr), s, i(+64*qc)]
        quads[0].append(np.transpose(ot[0:64, :, 0:64], (1, 2, 0)))
        quads[1].append(np.transpose(ot[0:64, :, 64:128], (1, 2, 0)))
        quads[2].append(np.transpose(ot[64:128, :, 0:64], (1, 2, 0)))
        quads[3].append(np.transpose(ot[64:128, :, 64:128], (1, 2, 0)))

    B, C, H, W = x.shape
    out = tuple(
        np.ascontiguousarray(np.concatenate(q, axis=0))
        .reshape(B, C, H // 2, W // 2)
        .astype(np.float32)
        for q in quads
    )
    return out, res.exec_time_ns


def kernel(x: np.ndarray, w_l: np.ndarray, w_h: np.ndarray):
    out, _ = run_on_hw(x, w_l, w_h, trace=False)
    return out
